# revision 31
# baseline (speedup 1.0000x reference)
"""KANLinear forward as a Bass/Tile kernel for 8 Trainium2 NeuronCores.

Math.  The reference's per-feature spline map S_io(x) is a piecewise cubic
with breakpoints thr1~0.2, thr2~0.6 (pieces t = idx-5 in {0,1,2} for
x in [0,1)).  Two structural facts collapse it to 8 matmul planes:

1. Across each knot the piece-difference polynomials Delta_q(x) of the six
   weight planes are all proportional to ONE fixed cubic phi(r), r = x-thr,
   with per-q ratios kappa = (1,-4,6,-4,1)/6 (a 4th-difference pattern from
   the reference's sliding-window basis gather).  So the entire knot
   correction is phi(r)*J_io with a single scalar J_io = sum_q kappa_q W_ioq.
2. Splitting phi(r) = phi(0) + psi(r) with psi(0) = 0, the correction
   becomes  phi(0)*g + psi(r)  where g = (x >= thr), and psi needs no mask
   (psi(relu(x-thr)) vanishes below the knot).

  y[n,o] = bias[o] + sum_i [ z c1 + z^2 c2 + z^3 c3          (z = x - 1/2)
                             + g1 e0_1 J1 + psi1(r) J1
                             + g2 e0_2 J2 + psi2(s) J2
                             + silu(x) W_base ]

Planes (f16): [z, g1, g2, silu, z2, z3, psi1, psi2]; accumulation in f32
PSUM; bias folded into the PSUM->SBUF evacuation.  The step planes g1/g2
and relu args are computed from the *f32* x (the map is discontinuous at
the knots, so classification must match the reference bit-exactly-ish;
psi is continuous so f16 is fine there).

Data-parallel over the batch: 16384 rows -> 8 shards of 2048.  Kernel
computes y^T [out, n] in f16; host transposes/casts back.  A jitted
shard_map runner is cached across calls so repeat invocations skip
retrace/recompile.
"""
import hashlib
import numpy as np
from contextlib import ExitStack

from concourse import bacc, tile, mybir
from concourse.bass_utils import run_bass_kernel_spmd

N_TOTAL, IN_F, OUT_F = 16384, 256, 256
N_CORES = 8
N_SHARD = N_TOTAL // N_CORES          # 2048
S, G = 3, 5
H32 = np.float32(0.4)
LO32 = np.float32(-1.0)
F32 = mybir.dt.float32
F16 = mybir.dt.float16

NUM_PLANES = 6
N_SUB = 512
N_SUBS = N_SHARD // N_SUB             # 4


def _basis_matrix():
    M = np.array([[1.0]], dtype=np.float32)
    scalar = 1.0
    for k in range(2, S + 2):
        t1 = np.pad(M, ((0, 1), (0, 0)))
        t3 = np.pad(M, ((1, 0), (0, 0)))
        t2 = np.zeros((k - 1, k), np.float32)
        t4 = np.zeros((k - 1, k), np.float32)
        for i in range(k - 1):
            t2[i, i] = i + 1
            t2[i, i + 1] = k - (i + 2)
            t4[i, i] = -1.0
            t4[i, i + 1] = 1.0
        M = t1 @ t2 + t3 @ t4
        scalar *= 1.0 / (k - 1)
    return (M * scalar).astype(np.float32)


def _piece_coeffs():
    """P[t, qi, p]: coefficient of x^p in basis_out[.., q=qi+2] on piece t."""
    B = _basis_matrix().astype(np.float64)
    h = np.float64(H32)
    P = np.zeros((3, 6, 4))
    for t in range(3):
        idx = t + 5
        fv = np.float64(np.float32(np.float32(idx) * H32 + LO32))
        u1c = np.array([-fv / h, 1.0 / h])  # u1 = u1c[0] + u1c[1]*x
        upow = [np.array([1.0]), u1c.copy()]
        for p in range(2, 4):
            c = np.zeros(p + 1)
            prev = upow[-1]
            c[: len(prev)] += prev * u1c[0]
            c[1 : len(prev) + 1] += prev * u1c[1]
            upow.append(c)
        for q in range(2, 8):
            j = q - 2 - t
            if 0 <= j <= 3:
                for p in range(4):
                    cc = upow[p]
                    P[t, q - 2, : len(cc)] += B[p, j] * cc
    grid1d = (np.arange(-S, G + S + 1, dtype=np.float32) * H32 + LO32).astype(np.float32)
    return P, np.float64(grid1d[6]), np.float64(grid1d[7])


_P, _THR1, _THR2 = _piece_coeffs()


def _taylor_at(poly_xpow, t):
    out = np.zeros(4)
    der = np.array(poly_xpow, dtype=np.float64)
    fact = 1.0
    for k in range(4):
        out[k] = np.polyval(der[::-1], t) / fact
        der = np.polyder(der[::-1])[::-1]
        fact *= (k + 1)
    return out


def _knot_decomp():
    """Per knot: phi (r-power coeffs of the rank-1 piece-difference) and
    kappa (per-q weight contraction pattern)."""
    knots = []
    for (ta, tb, thr) in ((0, 1, _THR1), (1, 2, _THR2)):
        D = _P[tb] - _P[ta]
        Dt = np.stack([_taylor_at(D[q], thr) for q in range(6)])
        qref = int(np.argmax(np.abs(Dt).sum(1)))
        phi = Dt[qref]
        kappa = (Dt @ phi) / (phi @ phi)
        knots.append((phi, kappa))
    return knots


(_PHI1, _KAP1), (_PHI2, _KAP2) = _knot_decomp()
# psi(r) = e1 r + e2 r^2 + e3 r^3 = r * (e3 (r+alpha)^2 + beta)
def _psi_consts(phi):
    e1, e2, e3 = phi[1:]
    return float(e2 / (2 * e3)), float(e1 - e2 * e2 / (4 * e3)), float(e3)


_ALPHA1, _BETA1, _E3_1 = _psi_consts(_PHI1)
_ALPHA2, _BETA2, _E3_2 = _psi_consts(_PHI2)
_BINOM = np.array([[1, 0, 0, 0], [0.5, 1, 0, 0], [0.25, 1, 1, 0], [0.125, 0.75, 1.5, 1]],
                  dtype=np.float64)  # x^p = sum_k BINOM[p,k] z^k, z = x-1/2


def pack_weights(weight):
    """weight [in,out,9] f32 -> (planes_w [6,in,out] f16, bias [out] f32).

    Plane order: [z, silu, z2, z3, psi'1, psi'2], where the merged knot plane
    psi'_k = phi_k(0)*g_k + psi_k(r) carries weight J_k (the step and psi
    weights are proportional, so one plane serves both)."""
    W = weight[:, :, 2:8].astype(np.float64)
    Ghat = np.einsum("ioq,tqp->tpio", W, _P)     # piece polys in x-powers
    cz = np.einsum("pio,pk->kio", Ghat[0], _BINOM)  # base poly in z-powers
    J1 = np.einsum("ioq,q->io", W, _KAP1)
    J2 = np.einsum("ioq,q->io", W, _KAP2)
    bias = cz[0].sum(axis=0)
    planes = np.stack([
        cz[1],                       # z
        weight[:, :, 8].astype(np.float64),  # silu
        cz[2],                       # z2
        cz[3],                       # z3
        J1,                          # psi'1
        J2,                          # psi'2
    ])
    return planes.astype(np.float16), bias.astype(np.float32)


_CACHE = {}


def _build_nc():
    nc = bacc.Bacc("TRN2", target_bir_lowering=False, debug=False)
    # x ships as f16 (halves the bandwidth-limited input DMA); the host
    # nudges boundary-straddling values by one ulp so the f16 comparisons
    # against thr1/thr2 classify exactly like the reference's f32 compare
    xt_d = nc.dram_tensor("xt", [IN_F, N_SHARD], F16, kind="ExternalInput").ap()
    # all 8x2 weight tiles packed column-wise into one tensor -> one DMA
    wall_d = nc.dram_tensor("wall", [128, NUM_PLANES * 2 * OUT_F], F16,
                            kind="ExternalInput").ap()
    bias_d = nc.dram_tensor("bias", [OUT_F, 1], F32, kind="ExternalInput").ap()
    yt_d = nc.dram_tensor("yt", [OUT_F, N_SHARD], F16, kind="ExternalOutput").ap()

    thr1, thr2 = float(_THR1), float(_THR2)
    mu = mybir.AluOpType.mult
    ge = mybir.AluOpType.is_ge
    ad = mybir.AluOpType.add
    mx = mybir.AluOpType.max
    AF = mybir.ActivationFunctionType
    NH = N_SHARD // 2                 # elementwise n-chunk (1024)

    with tile.TileContext(nc) as tc, ExitStack() as ctx:
        wpool = ctx.enter_context(tc.tile_pool(name="w", bufs=1))
        xpool = ctx.enter_context(tc.tile_pool(name="x", bufs=1))
        ppool = ctx.enter_context(tc.tile_pool(name="planes", bufs=1))
        opool = ctx.enter_context(tc.tile_pool(name="out", bufs=1))
        pspool = ctx.enter_context(tc.tile_pool(name="ps", bufs=1, space="PSUM"))

        # DMA trigger instructions cost ~650ns each and serialize per engine;
        # spread them across engines so all transfers start right after the
        # preamble.  wall is it-major so the it0 half unblocks matmuls early.
        X = [xpool.tile([128, N_SHARD], F16, name=f"x{it}", tag=f"x{it}")
             for it in range(2)]
        wall = wpool.tile([128, NUM_PLANES * 2 * OUT_F], F16, name="wall", tag="wall")
        WHALF = NUM_PLANES * OUT_F
        nc.scalar.dma_start(out=X[0][:, 0:NH], in_=xt_d[0:128, 0:NH])
        nc.gpsimd.dma_start(out=X[1][:, 0:NH], in_=xt_d[128:256, 0:NH])
        nc.sync.dma_start(out=wall[:, 0:WHALF], in_=wall_d[:, 0:WHALF])
        nc.sync.dma_start(out=wall[:, WHALF:2 * WHALF], in_=wall_d[:, WHALF:2 * WHALF])
        nc.gpsimd.dma_start(out=X[0][:, NH:N_SHARD], in_=xt_d[0:128, NH:N_SHARD])
        nc.gpsimd.dma_start(out=X[1][:, NH:N_SHARD], in_=xt_d[128:256, NH:N_SHARD])
        b_sb = [wpool.tile([128, 1], F32, name=f"b{ot}", tag=f"b{ot}") for ot in range(2)]
        for ot in range(2):
            nc.gpsimd.dma_start(out=b_sb[ot][:], in_=bias_d[ot * 128:(ot + 1) * 128, :])

        def wtile(p, it):
            base = (it * NUM_PLANES + p) * OUT_F
            return wall[:, base:base + OUT_F]

        # per-partition const tiles for activation biases
        cvals = {"a1": _ALPHA1, "a2": _ALPHA2}
        cb = {}
        for nm, v in cvals.items():
            tl = wpool.tile([128, 1], F32, name=f"c_{nm}", tag=f"c_{nm}")
            nc.vector.memset(tl[:], float(v))
            cb[nm] = tl

        planes = [[None] * NUM_PLANES for _ in range(2)]
        t_all = {}
        for it in range(2):
            t = {nm: ppool.tile([128, N_SHARD], F16, name=f"{nm}_{it}", tag=f"{nm}_{it}")
                 for nm in ("z", "g1", "g2", "sl", "z2", "z3", "psi1", "psi2",
                            "ps1p", "ps2p", "r", "s", "q1", "q2", "u1", "u2")}
            t_all[it] = t
            planes[it] = [t["z"], t["sl"], t["z2"], t["z3"], t["ps1p"], t["ps2p"]]
        # elementwise in column-halves so compute starts as soon as the first
        # x chunk lands; engines chosen from measured costs (DVE TS is 4x /
        # TT 2x on f16; ACT is ~(224+FD/2)/1.2 regardless)
        for c in range(2):
            cs = slice(c * NH, (c + 1) * NH)
            # 1-op planes for BOTH it halves first: they feed the first
            # matmul bank runs
            for it in range(2):
                t = t_all[it]
                Xc = X[it][:, cs]
                nc.vector.tensor_scalar(t["z"][:, cs], Xc, -0.5, None, ad)
                nc.vector.tensor_scalar(t["g1"][:, cs], Xc, thr1, None, ge)
                nc.vector.tensor_scalar(t["g2"][:, cs], Xc, thr2, None, ge)
            for it in range(2):
                t = t_all[it]
                nc.scalar.activation(t["sl"][:, cs], X[it][:, cs], AF.Silu)
            for it in range(2):
                t = t_all[it]
                Xc = X[it][:, cs]
                nc.vector.tensor_scalar(t["r"][:, cs], Xc, -thr1, 0.0, ad, mx)
                nc.vector.tensor_scalar(t["s"][:, cs], Xc, -thr2, 0.0, ad, mx)
            for it in range(2):
                t = t_all[it]
                nc.scalar.activation(t["z2"][:, cs], t["z"][:, cs], AF.Square)
                nc.scalar.activation(t["q1"][:, cs], t["r"][:, cs], AF.Square,
                                     bias=cb["a1"][:])
                nc.scalar.activation(t["q2"][:, cs], t["s"][:, cs], AF.Square,
                                     bias=cb["a2"][:])
            for it in range(2):
                t = t_all[it]
                # u = e3*q + beta on ACT (Copy with scale+bias immediates)
                nc.scalar.activation(t["u1"][:, cs], t["q1"][:, cs], AF.Copy,
                                     bias=_BETA1, scale=_E3_1)
                nc.scalar.activation(t["u2"][:, cs], t["q2"][:, cs], AF.Copy,
                                     bias=_BETA2, scale=_E3_2)
            for it in range(2):
                t = t_all[it]
                nc.vector.tensor_tensor(t["z3"][:, cs], t["z2"][:, cs],
                                        t["z"][:, cs], mu)
                nc.vector.tensor_tensor(t["psi1"][:, cs], t["u1"][:, cs],
                                        t["r"][:, cs], mu)
                # psi' = phi(0)*g + psi  (merged step+psi plane, weight J)
                nc.vector.scalar_tensor_tensor(
                    t["ps1p"][:, cs], t["g1"][:, cs], float(_PHI1[0]),
                    t["psi1"][:, cs], mu, ad)
                nc.vector.tensor_tensor(t["psi2"][:, cs], t["u2"][:, cs],
                                        t["s"][:, cs], mu)
                nc.vector.scalar_tensor_tensor(
                    t["ps2p"][:, cs], t["g2"][:, cs], float(_PHI2[0]),
                    t["psi2"][:, cs], mu, ad)

        # bank-major runs of 8 same-bank matmuls (half the planes per run) so
        # the PSUM queue doesn't cycle banks every instruction; first half is
        # the 1-op planes so the stream can start early
        ps = [[pspool.tile([128, N_SUB], F32, name=f"ps{ot}_{sb}", tag=f"ps{ot}_{sb}")
               for sb in range(N_SUBS)] for ot in range(2)]
        for half in (range(0, 3), range(3, NUM_PLANES)):
            for sb in range(N_SUBS):          # sb-major: c0 banks first
                for ot in range(2):
                    for it in range(2):       # it-major: it0 planes suffice early
                        for p in half:
                            lhsT = wtile(p, it)[:, ot * 128:(ot + 1) * 128]
                            rhs = planes[it][p][:, sb * N_SUB:(sb + 1) * N_SUB]
                            nc.tensor.matmul(
                                ps[ot][sb][:], lhsT, rhs,
                                start=(p == 0 and it == 0),
                                stop=(p == NUM_PLANES - 1 and it == 1))
        # per-bank evacuation + output DMA in bank-completion order (sb-major
        # matches the half1 matmul order) so the write-back overlaps the
        # matmul stream instead of trailing it
        yo = [opool.tile([128, N_SHARD], F16, name=f"yo{ot}", tag=f"yo{ot}")
              for ot in range(2)]
        for sb in range(N_SUBS):
            for ot in range(2):
                nc.scalar.activation(yo[ot][:, sb * N_SUB:(sb + 1) * N_SUB],
                                     ps[ot][sb][:], AF.Identity, bias=b_sb[ot][:])
                nc.scalar.dma_start(
                    out=yt_d[ot * 128:(ot + 1) * 128, sb * N_SUB:(sb + 1) * N_SUB],
                    in_=yo[ot][:, sb * N_SUB:(sb + 1) * N_SUB])
    nc.compile()
    return nc


def _get_nc():
    if "nc" not in _CACHE:
        _CACHE["nc"] = _build_nc()
    return _CACHE["nc"]


def _make_in_maps(x, weight):
    wkey = hashlib.blake2b(weight.tobytes(), digest_size=16).hexdigest()
    packed = _CACHE.get("packed")
    if packed is None or packed[0] != wkey:
        planes_w, bias = pack_weights(weight)
        # [8,256,256] -> [128, 2*8*256] column-blocked by (it, p)
        wall = np.ascontiguousarray(
            planes_w.reshape(NUM_PLANES, 2, 128, OUT_F)
            .transpose(2, 1, 0, 3).reshape(128, NUM_PLANES * 2 * OUT_F))
        base = {"bias": np.ascontiguousarray(bias[:, None]), "wall": wall}
        packed = (wkey, base)
        _CACHE["packed"] = packed
    base = packed[1]
    # f16 x with boundary-consistent nudging: flip the handful of values
    # whose f16 rounding crosses thr1/thr2 so device-side comparisons match
    # the reference's f32 classification (the map jumps at the knots)
    xh = x.astype(np.float16)
    back = xh.astype(np.float32)
    for thr in (np.float32(_THR1), np.float32(_THR2)):
        t16 = np.float16(thr)
        up = t16 if np.float32(t16) >= thr else np.nextafter(t16, np.float16(10))
        dn = np.nextafter(t16, np.float16(-10)) if np.float32(t16) >= thr else t16
        bad_hi = (x >= thr) & (back < thr)
        bad_lo = (x < thr) & (back >= thr)
        if bad_hi.any():
            xh[bad_hi] = up
        if bad_lo.any():
            xh[bad_lo] = dn
        back = xh.astype(np.float32)
    in_maps = []
    for cid in range(N_CORES):
        m = dict(base)
        m["xt"] = np.ascontiguousarray(xh[cid * N_SHARD:(cid + 1) * N_SHARD, :].T)
        in_maps.append(m)
    return in_maps


def _get_runner():
    """Cached jitted shard_map executor (mirrors run_bass_via_pjrt's multi-core
    path, but reuses the compiled executable across kernel() calls)."""
    if "runner" in _CACHE:
        return _CACHE["runner"]
    import jax
    import jax.numpy as jnp
    from jax.sharding import Mesh, PartitionSpec, NamedSharding
    from jax.experimental.shard_map import shard_map
    from concourse import bass2jax

    nc = _get_nc()
    bass2jax.install_neuronx_cc_hook()
    partition_name = nc.partition_id_tensor.name if nc.partition_id_tensor else None
    in_names, out_names, out_avals, zero_shapes = [], [], [], []
    for alloc in nc.m.functions[0].allocations:
        if not isinstance(alloc, mybir.MemoryLocationSet):
            continue
        name = alloc.memorylocations[0].name
        if alloc.kind == "ExternalInput":
            if name != partition_name:
                in_names.append(name)
        elif alloc.kind == "ExternalOutput":
            out_names.append(name)
            shape = tuple(alloc.tensor_shape)
            dtype = mybir.dt.np(alloc.dtype)
            out_avals.append(jax.core.ShapedArray(shape, dtype))
            zero_shapes.append((shape, dtype))
    n_params = len(in_names)
    n_outs = len(out_avals)
    all_in_names = in_names + out_names + ([partition_name] if partition_name else [])
    donate = tuple(range(n_params, n_params + n_outs))

    def _body(*args):
        operands = list(args)
        if partition_name is not None:
            operands.append(bass2jax.partition_id_tensor())
        outs = bass2jax._bass_exec_p.bind(
            *operands,
            out_avals=tuple(out_avals),
            in_names=tuple(all_in_names),
            out_names=tuple(out_names),
            lowering_input_output_aliases=(),
            sim_require_finite=True,
            sim_require_nnan=True,
            nc=nc,
        )
        return tuple(outs)

    devices = jax.devices()[:N_CORES]
    mesh = Mesh(np.asarray(devices), ("core",))
    in_specs = (PartitionSpec("core"),) * (n_params + n_outs)
    out_specs = (PartitionSpec("core"),) * n_outs
    sharded = jax.jit(
        shard_map(_body, mesh=mesh, in_specs=in_specs, out_specs=out_specs,
                  check_rep=False),
        donate_argnums=donate, keep_unused=True,
    )
    sharding = NamedSharding(mesh, PartitionSpec("core"))
    zeros_fn = jax.jit(
        lambda: tuple(jnp.zeros((N_CORES * s[0], *s[1:]), d) for s, d in zero_shapes),
        out_shardings=(sharding,) * n_outs)
    runner = {
        "jax": jax, "sharding": sharding, "sharded": sharded, "zeros_fn": zeros_fn,
        "in_names": in_names, "out_names": out_names, "out_avals": out_avals,
        "dev_cache": {},
    }
    _CACHE["runner"] = runner
    return runner


def _fast_call(in_maps):
    r = _get_runner()
    jax = r["jax"]
    concat_in = []
    for i, name in enumerate(r["in_names"]):
        if name == "xt":
            arr = np.concatenate([m["xt"] for m in in_maps], axis=0)
            concat_in.append(arr)
        else:
            # weights/bias are identical across cores and across repeat calls;
            # keep them device-resident keyed by content
            cached = r["dev_cache"].get(name)
            h = hashlib.blake2b(in_maps[0][name].tobytes(), digest_size=8).digest()
            if cached is None or cached[0] != h:
                arr = np.concatenate([m[name] for m in in_maps], axis=0)
                dev = jax.device_put(arr, r["sharding"])
                r["dev_cache"][name] = (h, dev)
                cached = (h, dev)
            concat_in.append(cached[1])
    zeros = r["zeros_fn"]()
    out_arrs = r["sharded"](*concat_in, *zeros)
    outs = []
    for c in range(N_CORES):
        outs.append({name: np.asarray(out_arrs[i]).reshape(
            N_CORES, *r["out_avals"][i].shape)[c]
            for i, name in enumerate(r["out_names"])})
    return outs


def kernel(x, weight):
    x = np.asarray(x, dtype=np.float32)
    weight = np.asarray(weight, dtype=np.float32)
    in_maps = _make_in_maps(x, weight)

    if _CACHE.get("trace"):
        res = run_bass_kernel_spmd(_get_nc(), in_maps, list(range(N_CORES)),
                                   trace=True)
        _CACHE["last_result"] = res
        results = res.results
    else:
        results = _fast_call(in_maps)

    out = np.concatenate([r["yt"].T for r in results], axis=0)
    return out.astype(np.float32)
